# revision 13
# baseline (speedup 1.0000x reference)
"""Multi-head causal attention (B=2, T=2048, C=1024, H=16, D=64) on 8 TRN2 cores.

Sharding: 2 heads per core (tensor-parallel over H). x replicated, shipped
twice: fp8e4m3 DoubleRow-packed (Q/K projections; W pre-scaled x64 to dodge
fp8 denormals, undone via the exp scale) and bf16 (V projection, so the
V-path carries no fp8 noise -- output errors on the V side do not average
down, fp8 there costs ~3% rel err).

Per-core dataflow:
  1. Q/K projections as fp8 DoubleRow matmuls (K=256/pass, 2x MACs/cycle);
     V projection in bf16 (moving) x f32r Wv (stationary). Q^T/K^T drain
     PSUM->SBUF as bf16 (values 64q, 64k), V^T as f32r (lossless).
  2. V^T PE-transposed (f32r) to V[s, d] per head with a 1.0 column
     appended (V_aug[s, 65]) so the AV matmul also emits softmax sums.
  3. Scores S^T[s, t] = K^T x Q^T per s-block in bf16; the two heads'
     matmuls run CONCURRENTLY via PE row tiling (K=64 each at row groups
     0/64). Causal column trim. PSUM value = 131072 * score.
  4. ONE exp per s-block covers both heads: e = exp(S/131072) -> bf16.
     The diagonal 128x128 gets a multiplicative 0/1 triangle on GPSIMD
     (e lives in SBUF; keeps DVE and the ACT critical path clear).
  5. AV: V_aug stationary (f32r), E^T moving (bf16, so 1 cycle/row at all
     causal-trimmed widths), accumulated over s-blocks in PSUM ->
     out^T[65, t]; row 64 = softmax sums.
  6. PE-transpose (f32r) to [t, 65], DVE reciprocal + per-partition scalar
     multiply, DMA out.

Schedule: one fused streaming pipeline per (b, t-tile); the NEXT tile's
projection work is interleaved into the current tile's s-block periods, and
AV lags scores by 2 s-blocks, so the PE never idles.
"""

import numpy as np

import concourse.mybir as mybir
import concourse.tile as tile
from concourse import bacc
from concourse.masks import make_identity

B, T, C, H, D = 2, 2048, 1024, 16, 64
HPC = 2          # heads per core
NCORES = 8
TT = 512         # t-tile (moving free dim)
SB = 128         # s-block (scores stationary free dim)
NCH = C // 128   # bf16 contraction chunks (V projection)
NKK = C // 256   # DoubleRow contraction chunks (Q/K projections)
F32 = mybir.dt.float32
F32R = mybir.dt.float32r
BF16 = mybir.dt.bfloat16
FP8 = mybir.dt.float8e4
DR = mybir.MatmulPerfMode.DoubleRow
SSCALE = 1.0 / 131072.0  # undo 64*64 weight prescale and apply C**-0.5


def build_nc(t_len=T, batches=B):
    nj = t_len // TT
    nc = bacc.Bacc("TRN2", target_bir_lowering=False, debug=False)
    x8 = nc.dram_tensor("x8", [batches, 128, NKK, 2, t_len], FP8,
                        kind="ExternalInput")
    xb = nc.dram_tensor("xb", [batches, 128, NCH, t_len], F32R,
                        kind="ExternalInput")
    wq = nc.dram_tensor("wq", [128, NKK, 2, 2 * D], FP8, kind="ExternalInput")
    wk = nc.dram_tensor("wk", [128, NKK, 2, 2 * D], FP8, kind="ExternalInput")
    wv = nc.dram_tensor("wv", [128, NCH, 2 * D], F32R, kind="ExternalInput")
    y = nc.dram_tensor("y", [batches, t_len, 2 * D], F32, kind="ExternalOutput")

    with tile.TileContext(nc) as tc:
        with (
            tc.tile_pool(name="consts", bufs=1) as consts,
            tc.tile_pool(name="wpool", bufs=1) as wpool,
            tc.tile_pool(name="qkv", bufs=batches) as qkv,
            tc.tile_pool(name="epool", bufs=4) as epool,
            tc.tile_pool(name="avs", bufs=2) as avs,
            tc.tile_pool(name="outp", bufs=8) as outp,
            tc.tile_pool(name="small", bufs=8) as small,
        ):
            idf = consts.tile([128, 128], F32)
            make_identity(nc, idf)
            id_bf16 = consts.tile([128, 128], BF16)
            nc.vector.tensor_copy(id_bf16, idf)
            # tri01[s, t_local] = 1 where t_local >= s else 0; multiplied
            # into the diagonal 128x128 sub-block of E after exp.
            tri01 = consts.tile([128, SB], F32R)
            nc.gpsimd.memset(tri01.bitcast(F32), 1.0)
            nc.gpsimd.affine_select(
                out=tri01.bitcast(F32), in_=tri01.bitcast(F32),
                compare_op=mybir.AluOpType.is_ge,
                fill=0.0, base=0,
                pattern=[[1, SB]], channel_multiplier=-1,
            )

            w_sb, w_src = {}, {}
            for name, w, shape, dt in (
                    ("q", wq, [128, NKK, 2, 2 * D], FP8),
                    ("k", wk, [128, NKK, 2, 2 * D], FP8),
                    ("v", wv, [128, NCH, 2 * D], F32R)):
                wt = wpool.tile(shape, dt, tag=f"w{name}", name=f"w{name}_sb")
                w_sb[name] = wt
                w_src[name] = w

            # Persistent per-batch tensors
            QT, KT, VH = {}, {}, {}
            for b in range(batches):
                QT[b] = qkv.tile([128, t_len], BF16, tag="qt", name=f"qt{b}")
                KT[b] = qkv.tile([128, t_len], BF16, tag="kt", name=f"kt{b}")
                for h in range(HPC):
                    vh = qkv.tile([128, t_len // SB, D + 1], BF16,
                                  tag=f"vh{h}", name=f"vh{b}_{h}")
                    nc.gpsimd.memset(vh[:, :, D:D + 1], 1.0)
                    VH[(b, h)] = vh

            # ---------------- fused streaming pipeline ----------------
            with (
                tc.tile_pool(name="xtp", bufs=3) as xtp,
                tc.tile_pool(name="vts", bufs=2) as vts,
                tc.tile_pool(name="mixps", bufs=2, space="PSUM") as mixps,
                tc.tile_pool(name="spsum", bufs=2, space="PSUM") as spsum,
                tc.tile_pool(name="avpsum", bufs=2, space="PSUM") as avpsum,
            ):
                def proj_closures(b, j):
                    """Projection work for (b, j) as a list of closures, to
                    be interleaved into the previous tile's attention
                    periods so neither PE nor ScalarE ever starves."""
                    state = {}

                    def do_load():
                        x8_sb = xtp.tile([128, NKK, 2, TT], FP8, tag="x8t",
                                         name=f"x8t{b}_{j}")
                        nc.sync.dma_start(
                            out=x8_sb,
                            in_=x8[b][:, :, :, j * TT:(j + 1) * TT])
                        xb_sb = xtp.tile([128, NCH, TT], F32R, tag="xbt",
                                         name=f"xbt{b}_{j}")
                        half = NCH // 2
                        nc.sync.dma_start(
                            out=xb_sb[:, 0:half, :],
                            in_=xb[b][:, 0:half, j * TT:(j + 1) * TT])
                        nc.sync.dma_start(
                            out=xb_sb[:, half:, :],
                            in_=xb[b][:, half:, j * TT:(j + 1) * TT])
                        state["x8"] = x8_sb
                        state["xb"] = xb_sb

                    def do_proj(name):
                        pp = mixps.tile([128, TT], F32, tag="mix",
                                        name=f"pp_{name}")
                        if name == "v":
                            for kk in range(NCH):
                                nc.tensor.matmul(
                                    pp,
                                    lhsT=w_sb["v"][:, kk, :],
                                    rhs=state["xb"][:, kk, :],
                                    start=(kk == 0), stop=(kk == NCH - 1),
                                    skip_group_check=True,
                                )
                            vt_sb = vts.tile([128, TT], BF16, tag="vt",
                                             name=f"vt{b}_{j}")
                            nc.vector.tensor_copy(vt_sb, pp)
                            state["vt"] = vt_sb
                        else:
                            for kk in range(NKK):
                                nc.tensor.matmul(
                                    pp,
                                    lhsT=w_sb[name][:, kk, :, :],
                                    rhs=state["x8"][:, kk, :, :],
                                    start=(kk == 0), stop=(kk == NKK - 1),
                                    perf_mode=DR,
                                    skip_group_check=True,
                                )
                            dst = QT[b] if name == "q" else KT[b]
                            nc.vector.tensor_copy(
                                dst[:, j * TT:(j + 1) * TT], pp)

                    def do_vtrans(q4):
                        vp = mixps.tile([128, 128], BF16, tag="mix",
                                        name=f"vp{q4}")
                        nc.tensor.transpose(
                            vp, state["vt"][:, q4 * 128:(q4 + 1) * 128],
                            id_bf16)
                        sb = (j * TT) // SB + q4
                        for h in range(HPC):
                            nc.vector.tensor_copy(
                                VH[(b, h)][:, sb, 0:D],
                                vp[:, h * D:(h + 1) * D])

                    ops = [lambda: do_proj("q"),
                           lambda: do_proj("k"),
                           lambda: do_proj("v")]
                    ops += [lambda q4=q4: do_vtrans(q4)
                            for q4 in range(TT // 128)]
                    return do_load, ops

                def emit_attention(b, j, pending):
                    """Causal attention for t-tile j. Per s-block: both
                    heads' score MMs row-tiled concurrent, ONE exp call for
                    both heads -> bf16 E, gpsimd tri-mask on the diagonal
                    block, AV lagging 2 s-blocks. Closures in `pending`
                    (next tile's projections) drain across the periods."""
                    out_tiles = [outp.tile([128, 2 * D], F32, tag="out",
                                           name=f"out{b}_{j}_{q}")
                                 for q in range(TT // 128)]
                    n_sb = (j + 1) * TT // SB
                    av_ps = {h: avpsum.tile([D + 1, TT], F32, tag="avps",
                                            name=f"avps{h}")
                             for h in range(HPC)}
                    eg = {}
                    LAG = 2

                    def emit_scores(sb):
                        off = max(0, (sb - 4 * j) * SB)
                        S = spsum.tile([128, HPC, TT], F32,
                                       tag="spsum", name=f"s{sb}")
                        for h in range(HPC):
                            hp = slice(h * D, (h + 1) * D)
                            nc.tensor.matmul(
                                S[:, h, off:TT],
                                lhsT=KT[b][hp, sb * SB:(sb + 1) * SB],
                                rhs=QT[b][hp, j * TT + off:(j + 1) * TT],
                                start=True, stop=True,
                            )
                        e = epool.tile([128, HPC, TT], BF16, tag="e",
                                       name=f"e{sb}")
                        nc.scalar.activation(
                            out=e[:, :, off:TT], in_=S[:, :, off:TT],
                            func=mybir.ActivationFunctionType.Exp,
                            scale=SSCALE)
                        if sb >= 4 * j:  # diagonal triangle at [off, off+SB)
                            for h in range(HPC):
                                nc.gpsimd.tensor_mul(
                                    e[:, h, off:off + SB],
                                    e[:, h, off:off + SB],
                                    tri01)
                        eg[sb] = (e, off)

                    def emit_av(sb):
                        e, off = eg.pop(sb)
                        for h in range(HPC):
                            nc.tensor.matmul(
                                av_ps[h][:, off:],
                                lhsT=VH[(b, h)][:, sb, :],
                                rhs=e[:, h, off:TT],
                                start=(sb == 0), stop=(sb == n_sb - 1),
                                skip_group_check=True,
                            )

                    n_periods = n_sb + LAG
                    n_pend = len(pending)
                    popped = 0
                    for sb in range(n_periods):
                        if sb < n_sb:
                            emit_scores(sb)
                        want = (n_pend * (sb + 1)) // n_periods
                        while popped < want:
                            pending[popped]()
                            popped += 1
                        if sb >= LAG:
                            emit_av(sb - LAG)
                    assert popped == n_pend

                    for h in range(HPC):
                        av_sb = avs.tile([D + 1, TT], F32, name=f"avsb{h}")
                        nc.vector.tensor_copy(av_sb, av_ps[h])
                        for q4 in range(TT // 128):
                            ot = mixps.tile([128, D + 1], F32, tag="mix",
                                            name=f"ot{h}_{q4}")
                            nc.tensor.transpose(
                                ot, av_sb[:, q4 * 128:(q4 + 1) * 128],
                                idf[0:D + 1, 0:D + 1])
                            rec = small.tile([128, 1], F32)
                            nc.vector.reciprocal(rec, ot[:, D:D + 1])
                            nc.vector.tensor_scalar_mul(
                                out_tiles[q4][:, h * D:(h + 1) * D],
                                ot[:, 0:D], rec)
                    for q4 in range(TT // 128):
                        t0 = j * TT + q4 * 128
                        nc.sync.dma_start(
                            out=y[b, t0:t0 + 128, :], in_=out_tiles[q4])

                seq = [(b, j) for b in range(batches) for j in range(nj)]
                # First tile's x loads go to the FRONT of the sync DMA
                # queue (before the W loads) so the first projection matmul
                # is gated only by its own transfers.
                ld0, ops0 = proj_closures(*seq[0])
                ld0()
                nc.sync.dma_start(out=w_sb["q"], in_=wq[:, :, :, :])
                nc.sync.dma_start(out=w_sb["k"], in_=wk[:, :, :, :])
                nc.sync.dma_start(out=w_sb["v"], in_=wv[:, :, :])
                for op in ops0:
                    op()
                for idx, (b, j) in enumerate(seq):
                    if idx + 1 < len(seq):
                        ldn, opsn = proj_closures(*seq[idx + 1])
                        nxt = [ldn] + opsn
                    else:
                        nxt = []
                    emit_attention(b, j, nxt)

    nc.compile()
    return nc


_CACHE = {}


def _get_runner():
    if "run" in _CACHE:
        return _CACHE["run"]

    import jax
    from jax.experimental.shard_map import shard_map
    from jax.sharding import Mesh, PartitionSpec
    from concourse import bass2jax
    from concourse.bass2jax import _bass_exec_p, install_neuronx_cc_hook

    nc = build_nc()
    install_neuronx_cc_hook()

    partition_name = (nc.partition_id_tensor.name
                      if nc.partition_id_tensor else None)
    in_names, out_names, out_avals, zero_outs = [], [], [], []
    for alloc in nc.m.functions[0].allocations:
        if not isinstance(alloc, mybir.MemoryLocationSet):
            continue
        name = alloc.memorylocations[0].name
        if alloc.kind == "ExternalInput":
            if name != partition_name:
                in_names.append(name)
        elif alloc.kind == "ExternalOutput":
            out_names.append(name)
            shape = tuple(alloc.tensor_shape)
            dtype = mybir.dt.np(alloc.dtype)
            out_avals.append(jax.core.ShapedArray(shape, dtype))
            zero_outs.append(np.zeros(shape, dtype))
    n_params = len(in_names)
    n_outs = len(out_avals)
    all_names = in_names + out_names
    if partition_name is not None:
        all_names = all_names + [partition_name]
    donate = tuple(range(n_params, n_params + n_outs))

    def _body(*args):
        operands = list(args)
        if partition_name is not None:
            operands.append(bass2jax.partition_id_tensor())
        outs = _bass_exec_p.bind(
            *operands,
            out_avals=tuple(out_avals),
            in_names=tuple(all_names),
            out_names=tuple(out_names),
            lowering_input_output_aliases=(),
            sim_require_finite=True,
            sim_require_nnan=True,
            nc=nc,
        )
        return tuple(outs)

    devices = jax.devices()[:NCORES]
    mesh = Mesh(np.asarray(devices), ("core",))
    in_specs = (PartitionSpec("core"),) * (n_params + n_outs)
    out_specs = (PartitionSpec("core"),) * n_outs
    sharded = jax.jit(
        shard_map(_body, mesh=mesh, in_specs=in_specs, out_specs=out_specs,
                  check_rep=False),
        donate_argnums=donate, keep_unused=True,
    )

    runner = {
        "nc": nc,
        "all_names": all_names,
        "sharded": sharded,
        "in_names": in_names,
        "out_names": out_names,
        "out_avals": out_avals,
        "zero_outs": zero_outs,
    }
    _CACHE["run"] = runner
    return runner


def _to8(a):
    return np.ascontiguousarray(
        np.clip(a, -240.0, 240.0).astype(mybir.dt.np(FP8)))


def _shard_inputs(x, Wq, Wk, Wv):
    """Per-core input dicts. Host-side layout prep only."""
    xt = np.transpose(x, (0, 2, 1)).astype(np.float32)  # [B, C, T]
    # fp8 DoubleRow layout: x8[b, p, kk, i, t] = xt[b, (kk*2+i)*128+p, t]
    x8 = _to8(np.transpose(
        xt.reshape(B, NKK, 2, 128, T), (0, 3, 1, 2, 4)))
    # bf16 layout for the V projection: xb[b, p, k, t] = xt[b, k*128+p, t]
    xb = np.ascontiguousarray(np.transpose(
        xt.reshape(B, NCH, 128, T), (0, 2, 1, 3)).astype(np.float32))

    def packw8(w, h0):
        w2 = np.concatenate([w[h0 + i] for i in range(HPC)],
                            axis=1) * 64.0  # [C, 2D]
        return _to8(np.transpose(
            w2.reshape(NKK, 2, 128, 2 * D), (2, 0, 1, 3)))

    maps = []
    for c in range(NCORES):
        h0 = HPC * c
        wv2 = np.concatenate([Wv[h0 + i] for i in range(HPC)], axis=1)
        maps.append({
            "x8": x8,
            "xb": xb,
            "wq": packw8(Wq, h0),
            "wk": packw8(Wk, h0),
            "wv": np.ascontiguousarray(np.transpose(
                wv2.reshape(NCH, 128, 2 * D),
                (1, 0, 2)).astype(np.float32)),
        })
    return maps


def run_sharded(in_maps):
    """Run the 8-core NEFF once; returns list of per-core output dicts."""
    r = _get_runner()
    concat_in = [
        np.concatenate([in_maps[c][name] for c in range(NCORES)], axis=0)
        for name in r["in_names"]
    ]
    concat_zeros = [
        np.zeros((NCORES * z.shape[0], *z.shape[1:]), z.dtype)
        for z in r["zero_outs"]
    ]
    out_arrs = r["sharded"](*concat_in, *concat_zeros)
    return [
        {
            name: np.asarray(out_arrs[i]).reshape(
                NCORES, *r["out_avals"][i].shape)[c]
            for i, name in enumerate(r["out_names"])
        }
        for c in range(NCORES)
    ]


def kernel(x, Wq, Wk, Wv):
    in_maps = _shard_inputs(
        np.asarray(x, dtype=np.float32), np.asarray(Wq, dtype=np.float32),
        np.asarray(Wk, dtype=np.float32), np.asarray(Wv, dtype=np.float32))
    results = run_sharded(in_maps)
    return np.concatenate([results[c]["y"] for c in range(NCORES)], axis=2)
